# revision 1
# baseline (speedup 1.0000x reference)
"""Distributed attention kernel for 8 trn2 NeuronCores.

Reference semantics (B=2, S=2048, D=2048, H=16, dh=128):
  q = x@W_q, k = x@W_k  (per-head split), v = x@W_v (full width)
  scores = q@k^T per head; (scores + triu(-1e9)) * 1/sqrt(dh); softmax
  out = (sum_h probs_h) @ v @ W_o        <- heads summed, v full width

Sharding: 2 groups of 4 cores (batch parallel); within a group, rank r
owns heads {4r..4r+3} (cols of W_q/W_k), cols [512r, 512r+512) of W_v.
Each core computes P_local = sum of its 4 heads' probs [2048, 2048],
ReduceScatter(add) over the group sums heads and shards rows: rank r
gets q-tiles {r, 4+r, 8+r, 12+r} (one per 4-tile slab). v is
AllGathered (bf16). Each core then computes Y = (P_own @ v) @ W_o for
its 512 q rows -> no second collective; host concatenates.

Precision: score path (x@Wq, x@Wk, q@k^T) in float32r (TF32-like
1-8-11, full PE rate); softmax in f32; P through RS in f32; v/P^T/O/W_o
in bf16 with f32 PSUM accumulation.
"""

import math

import numpy as np
import ml_dtypes

import concourse.bass as bass
import concourse.mybir as mybir
import concourse.tile as tile
from concourse import bacc
from concourse.bass_utils import run_bass_kernel_spmd
from concourse.masks import make_identity

F32 = mybir.dt.float32
F32R = mybir.dt.float32r
BF16 = mybir.dt.bfloat16

S = 2048
D = 2048
DH = 128
NT = S // 128  # 16 q/k tiles
SCALE = 1.0 / math.sqrt(DH)
GROUPS = [[0, 1, 2, 3], [4, 5, 6, 7]]
NEG = -1e9


def build():
    nc = bacc.Bacc("TRN2", target_bir_lowering=False, debug=False, num_devices=8)

    x = nc.declare_dram_parameter("x", [D, S], F32R, isOutput=False)  # x TRANSPOSED on host
    xbf = nc.declare_dram_parameter("xbf", [D, S], BF16, isOutput=False)
    wq = nc.declare_dram_parameter("wq", [D, 512], F32R, isOutput=False)
    wk = nc.declare_dram_parameter("wk", [D, 512], F32R, isOutput=False)
    wv = nc.declare_dram_parameter("wv", [D, 512], BF16, isOutput=False)
    wo = nc.declare_dram_parameter("wo", [D, D], BF16, isOutput=False)
    out = nc.declare_dram_parameter("out", [512, D], F32, isOutput=True)

    p_dram = nc.dram_tensor("p_dram", [S, S], BF16)
    p_own = nc.dram_tensor("p_own", [4, 128, S], BF16)
    v_local = nc.dram_tensor("v_local", [S, 512], BF16)
    v_ag = nc.dram_tensor("v_ag", [4, S, 512], BF16)

    with tile.TileContext(nc) as tc:
        with tc.tile_pool(name="const", bufs=1) as cst:
            ident = cst.tile([128, 128], F32)
            make_identity(nc, ident)
            ident_bf = cst.tile([128, 128], BF16)
            nc.vector.tensor_copy(out=ident_bf[:], in_=ident[:])
            zero = cst.tile([128, 512], BF16)
            nc.vector.memset(zero[:], 0.0)
            # mask variant m: [128, 512], 0 where col <= row + 128*m else -1e9
            masks = cst.tile([128, 4, 512], BF16)
            for m in range(4):
                nc.gpsimd.memset(masks[:, m, :], 0.0)
                nc.gpsimd.affine_select(
                    out=masks[:, m, :],
                    in_=masks[:, m, :],
                    compare_op=mybir.AluOpType.is_ge,
                    fill=NEG,
                    base=128 * m,
                    pattern=[[-1, 512]],
                    channel_multiplier=1,
                )
            # pt spans phases C (transpose-loads) and D (OT reads)
            pt = cst.tile([128, NT, 512], BF16)  # [k-part, kt, own-q-col]
            # ---------------- Phases A+B, S processed in halves ----------------
            with (
                tc.tile_pool(name="xt_pool", bufs=1) as xtp,
                tc.tile_pool(name="qk_pool", bufs=1) as qkp,
            ):
                qT = qkp.tile([128, 4, S], F32R)  # [dh-part, head, q]
                kT = qkp.tile([128, 4, S], F32R)
                with (
                    tc.tile_pool(name="xin", bufs=2) as xin,
                    tc.tile_pool(name="wtile", bufs=3) as wtp,
                    tc.tile_pool(name="vsb", bufs=2) as vsbp,
                    tc.tile_pool(name="ab_ps", bufs=8, space="PSUM") as pjp,
                ):
                    for sh in range(2):  # S half
                        s0 = sh * 1024
                        xt = xtp.tile([128, NT, 1024], F32R, tag="xt", name=f"xt{sh}")
                        first = True
                        for dst, wsrc in ((qT, wq), (kT, wk)):
                            psums = [
                                pjp.tile([128, 512], F32, tag="ps512", name=f"proj{_j}")
                                for _j in range(8)
                            ]
                            for Dt in range(NT):
                                if first:
                                    nc.sync.dma_start(
                                        xt[:, Dt, :],
                                        x[Dt * 128 : (Dt + 1) * 128, s0 : s0 + 1024],
                                    )
                                w_t = wtp.tile([128, 512], F32R, tag="w")
                                nc.sync.dma_start(
                                    w_t[:], wsrc[Dt * 128 : (Dt + 1) * 128, :]
                                )
                                for j in range(8):
                                    dt, qc = divmod(j, 2)
                                    nc.tensor.matmul(
                                        psums[j][:],
                                        w_t[:, dt * 128 : (dt + 1) * 128],
                                        xt[:, Dt, qc * 512 : (qc + 1) * 512],
                                        start=(Dt == 0),
                                        stop=(Dt == NT - 1),
                                    )
                            for j in range(8):
                                dt, qc = divmod(j, 2)
                                nc.vector.tensor_copy(
                                    out=dst[:, dt, s0 + qc * 512 : s0 + (qc + 1) * 512],
                                    in_=psums[j][:],
                                )
                            first = False
                        # -- B: v for k-tiles in this half --
                        psums = [
                            pjp.tile([128, 512], F32, tag="ps512", name=f"projv{_j}")
                            for _j in range(8)
                        ]
                        for Dt in range(NT):
                            xb_t = wtp.tile([128, 1024], BF16, tag="xb")
                            nc.sync.dma_start(
                                xb_t[:],
                                xbf[Dt * 128 : (Dt + 1) * 128, s0 : s0 + 1024],
                            )
                            w_t = wtp.tile([128, 512], BF16, tag="wv")
                            nc.sync.dma_start(w_t[:], wv[Dt * 128 : (Dt + 1) * 128, :])
                            for j in range(8):
                                nc.tensor.matmul(
                                    psums[j][:],
                                    xb_t[:, j * 128 : (j + 1) * 128],
                                    w_t[:],
                                    start=(Dt == 0),
                                    stop=(Dt == NT - 1),
                                )
                        for j in range(8):
                            kt = sh * 8 + j
                            v_sb = vsbp.tile([128, 512], BF16, tag="vsb")
                            nc.vector.tensor_copy(out=v_sb[:], in_=psums[j][:])
                            nc.sync.dma_start(
                                v_local[kt * 128 : (kt + 1) * 128, :], v_sb[:]
                            )
                nc.gpsimd.collective_compute(
                    "AllGather",
                    mybir.AluOpType.bypass,
                    ins=[v_local[:]],
                    outs=[v_ag[:]],
                    replica_groups=GROUPS,
                )

                # ---------------- Phase C: scores / softmax / P ----------------
                with (
                    tc.tile_pool(name="epool", bufs=3) as ep,
                    tc.tile_pool(name="small", bufs=32) as smp,
                    tc.tile_pool(name="dsm", bufs=12) as dsm,
                    tc.tile_pool(name="psb", bufs=6) as psbp,
                    tc.tile_pool(name="sc_ps", bufs=3, space="PSUM") as scp,
                    tc.tile_pool(name="p_ps", bufs=2, space="PSUM") as ppp,
                ):
                    i_order = [i for sl in (3, 2, 1, 0) for i in range(4 * sl, 4 * sl + 4)]
                    slab_done = {0: 0, 1: 0, 2: 0, 3: 0}
                    for i in i_order:
                        kwc = i // 4 + 1  # number of 512-wide k chunks
                        kw = kwc * 512
                        e_t = ep.tile([128, 4, 2048], BF16, tag="E")
                        d_hs = []
                        ntile = (kw + 1023) // 1024
                        dtid = (kw - 512) // 1024
                        doff = (kw - 512) % 1024
                        for h in range(4):
                            s_tiles = [
                                scp.tile([128, 1024], F32, tag="S", name=f"sch{_j}")
                                for _j in range(ntile)
                            ]
                            for kc in range(kwc):
                                nc.tensor.matmul(
                                    s_tiles[kc // 2][:, (kc % 2) * 512 : (kc % 2) * 512 + 512],
                                    qT[:, h, i * 128 : (i + 1) * 128],
                                    kT[:, h, kc * 512 : (kc + 1) * 512],
                                    start=True,
                                    stop=True,
                                )
                            m0 = (i % 4) * 128
                            nc.vector.tensor_tensor(
                                out=s_tiles[dtid][:, doff + m0 : doff + 512],
                                in0=s_tiles[dtid][:, doff + m0 : doff + 512],
                                in1=masks[:, i % 4, m0:512],
                                op=mybir.AluOpType.add,
                            )
                            mx = None
                            for t in range(ntile):
                                w = min(kw - 1024 * t, 1024)
                                mxt = smp.tile([128, 1], F32, tag="mx")
                                nc.vector.reduce_max(
                                    out=mxt[:],
                                    in_=s_tiles[t][:, :w],
                                    axis=mybir.AxisListType.X,
                                )
                                if mx is None:
                                    mx = mxt
                                else:
                                    mxn = smp.tile([128, 1], F32, tag="mx")
                                    nc.vector.tensor_tensor(
                                        out=mxn[:],
                                        in0=mx[:],
                                        in1=mxt[:],
                                        op=mybir.AluOpType.max,
                                    )
                                    mx = mxn
                            nmS = smp.tile([128, 1], F32, tag="mx")
                            nc.vector.tensor_scalar_mul(nmS[:], mx[:], -SCALE)
                            rtot = None
                            for t in range(ntile):
                                w = min(kw - 1024 * t, 1024)
                                rc = smp.tile([128, 1], F32, tag="mx")
                                nc.scalar.activation(
                                    out=e_t[:, h, 1024 * t : 1024 * t + w],
                                    in_=s_tiles[t][:, :w],
                                    func=mybir.ActivationFunctionType.Exp,
                                    bias=nmS[:],
                                    scale=SCALE,
                                    accum_out=rc[:],
                                )
                                if rtot is None:
                                    rtot = rc
                                else:
                                    rn = smp.tile([128, 1], F32, tag="mx")
                                    nc.vector.tensor_tensor(
                                        out=rn[:],
                                        in0=rtot[:],
                                        in1=rc[:],
                                        op=mybir.AluOpType.add,
                                    )
                                    rtot = rn
                            rinv = smp.tile([128, 1], F32, tag="mx")
                            nc.vector.reciprocal(out=rinv[:], in_=rtot[:])
                            d_h = dsm.tile([128, 128], BF16, tag="D")
                            nc.vector.tensor_scalar_mul(d_h[:], ident_bf[:], rinv[:])
                            d_hs.append(d_h)
                        for kc in range(kwc):
                            p_t = ppp.tile([128, 512], F32, tag="P")
                            for h in range(4):
                                nc.tensor.matmul(
                                    p_t[:],
                                    d_hs[h][:],
                                    e_t[:, h, kc * 512 : (kc + 1) * 512],
                                    start=(h == 0),
                                    stop=(h == 3),
                                )
                            pc = psbp.tile([128, 512], BF16, tag="psb")
                            nc.scalar.copy(out=pc[:], in_=p_t[:])
                            nc.sync.dma_start(
                                p_dram[
                                    i * 128 : (i + 1) * 128,
                                    kc * 512 : (kc + 1) * 512,
                                ],
                                pc[:],
                            )
                        for kc in range(kwc, 4):
                            nc.sync.dma_start(
                                p_dram[
                                    i * 128 : (i + 1) * 128,
                                    kc * 512 : (kc + 1) * 512,
                                ],
                                zero[:],
                            )
                        slab_done[i // 4] += 1
                        if slab_done[i // 4] == 4:
                            s_idx = i // 4
                            nc.gpsimd.collective_compute(
                                "ReduceScatter",
                                mybir.AluOpType.add,
                                ins=[p_dram[s_idx * 512 : (s_idx + 1) * 512, :]],
                                outs=[p_own[s_idx]],
                                replica_groups=GROUPS,
                            )
                            # transpose-load this slab's P rows: pt[kt, own-col]
                            for kt in range(4 * s_idx + 4):
                                nc.sync.dma_start_transpose(
                                    pt[:, kt, s_idx * 128 : (s_idx + 1) * 128],
                                    p_own[s_idx][:, kt * 128 : (kt + 1) * 128],
                                )

            # ---------------- Phase D: OT, Y ----------------
            with (
                tc.tile_pool(name="dpool", bufs=1) as dp,
                tc.tile_pool(name="ysb", bufs=2) as ysbp,
                tc.tile_pool(name="vfs", bufs=10) as vfsp,
            ):
                wo_sb = dp.tile([128, NT, D], BF16)
                for dt in range(NT):
                    nc.sync.dma_start(
                        wo_sb[:, dt, :], wo[dt * 128 : (dt + 1) * 128, :]
                    )
                ot = dp.tile([128, NT, 512], BF16)  # [dv-part, dvt, own-q]
                with (
                    tc.tile_pool(name="ot_ps", bufs=4, space="PSUM") as otbp,
                    tc.tile_pool(name="y_ps", bufs=4, space="PSUM") as yps,
                ):
                    for dvt in range(NT):
                        vf_t = vfsp.tile([128, NT, 128], BF16, tag="vf")
                        vsrc = v_ag[dvt // 4].rearrange("(t p) d -> p t d", p=128)
                        nc.sync.dma_start(
                            vf_t[:],
                            vsrc[:, :, (dvt % 4) * 128 : (dvt % 4) * 128 + 128],
                        )
                        po = otbp.tile([128, 512], F32, tag="OTB")
                        for kt in range(NT):
                            c0 = 128 * (kt // 4)
                            nc.tensor.matmul(
                                po[:, c0:512],
                                vf_t[:, kt, :],
                                pt[:, kt, c0:512],
                                start=(kt == 0),
                                stop=(kt == NT - 1),
                            )
                        nc.vector.tensor_copy(out=ot[:, dvt, :], in_=po[:])
                    for qb in range(4):
                        for nch in range(4):
                            yp = yps.tile([128, 512], F32, tag="Y")
                            for dvt in range(NT):
                                nc.tensor.matmul(
                                    yp[:],
                                    ot[:, dvt, qb * 128 : (qb + 1) * 128],
                                    wo_sb[:, dvt, nch * 512 : nch * 512 + 512],
                                    start=(dvt == 0),
                                    stop=(dvt == NT - 1),
                                )
                            y_sb = ysbp.tile([128, 512], F32, tag="ysb")
                            nc.scalar.copy(out=y_sb[:], in_=yp[:])
                            nc.sync.dma_start(
                                out[
                                    qb * 128 : (qb + 1) * 128,
                                    nch * 512 : nch * 512 + 512,
                                ],
                                y_sb[:],
                            )

    nc.compile()
    return nc


_NC_CACHE = None


def kernel(x, W_q, W_k, W_v, W_o):
    global _NC_CACHE
    x = np.asarray(x, dtype=np.float32)
    W_q = np.asarray(W_q, dtype=np.float32)
    W_k = np.asarray(W_k, dtype=np.float32)
    W_v = np.asarray(W_v, dtype=np.float32)
    W_o = np.asarray(W_o, dtype=np.float32)
    if _NC_CACHE is None:
        _NC_CACHE = build()
    nc = _NC_CACHE

    wo_bf = W_o.astype(ml_dtypes.bfloat16)
    xT = [np.ascontiguousarray(x[g].T) for g in range(2)]
    xT_bf = [t.astype(ml_dtypes.bfloat16) for t in xT]
    in_maps = []
    for c in range(8):
        g, r = divmod(c, 4)
        in_maps.append(
            {
                "x": xT[g],
                "xbf": xT_bf[g],
                "wq": np.ascontiguousarray(W_q[:, 512 * r : 512 * (r + 1)]),
                "wk": np.ascontiguousarray(W_k[:, 512 * r : 512 * (r + 1)]),
                "wv": np.ascontiguousarray(W_v[:, 512 * r : 512 * (r + 1)]).astype(ml_dtypes.bfloat16),
                "wo": wo_bf,
            }
        )
    res = run_bass_kernel_spmd(nc, in_maps, core_ids=list(range(8)))
    Y = np.empty((2, S, D), dtype=np.float32)
    for c in range(8):
        g, r = divmod(c, 4)
        o = res.results[c]["out"]
        for s_idx in range(4):
            t = 4 * s_idx + r
            Y[g, t * 128 : (t + 1) * 128, :] = o[s_idx * 128 : (s_idx + 1) * 128, :]
    return Y



# revision 17
# speedup vs baseline: 1.0140x; 1.0140x over previous
"""Distributed attention kernel for 8 trn2 NeuronCores (v2).

Reference semantics (B=2, S=2048, D=2048, H=16, dh=128):
  q = x@W_q, k = x@W_k  (per-head split), v = x@W_v (full width)
  scores = q@k^T per head; (scores + triu(-1e9)) * 1/sqrt(dh); softmax
  out = (sum_h probs_h) @ v @ W_o        <- heads summed, v full width

Sharding: 2 groups of 4 cores (batch parallel); within a group, rank r
owns heads {4r..4r+3} (cols of W_q/W_k), cols [512r, 512r+512) of W_v.
Each core computes P_local = sum of its 4 heads' probs per q-slab
(slab s = q tiles 4s..4s+3, width-trimmed to kw=512(s+1) causal cols).
AllToAll(slab) redistributes so rank r holds all 4 partials for q-tile
4s+r; vector-sums them, PE-transposes to P^T, computes
OT[dv, own q] = v^T P^T (v AllGathered bf16), Y = OT^T @ W_o at the end.

Perf notes vs v1: per-slab A2A (copy-rate) replaces ReduceScatter of
zero-padded [512,2048]; OT/Y pipelined between score slabs instead of a
serial tail; weights SBUF-resident in projections; dense tensor-queue
issue order to keep the PE activity monitor (HAM) at full clock.
"""

import math

import numpy as np
import ml_dtypes

import concourse.bass as bass
import concourse.mybir as mybir
import concourse.tile as tile
from concourse import bacc
from concourse.bass_utils import run_bass_kernel_spmd
from concourse.masks import make_identity

F32 = mybir.dt.float32
F32R = mybir.dt.float32r
BF16 = mybir.dt.bfloat16

S = 2048
D = 2048
DH = 128
NT = S // 128  # 16 q/k tiles
SCALE = 1.0 / math.sqrt(DH)
GROUPS = [[0, 1, 2, 3], [4, 5, 6, 7]]
NEG = -1e9


def build():
    nc = bacc.Bacc("TRN2", target_bir_lowering=False, debug=False, num_devices=8)

    x = nc.declare_dram_parameter("x", [D, S], F32R, isOutput=False)  # x^T
    xbf = nc.declare_dram_parameter("xbf", [D, S], BF16, isOutput=False)
    wq = nc.declare_dram_parameter("wq", [D, 512], F32R, isOutput=False)
    wk = nc.declare_dram_parameter("wk", [D, 512], F32R, isOutput=False)
    wv = nc.declare_dram_parameter("wv", [D, 512], BF16, isOutput=False)
    wo = nc.declare_dram_parameter("wo", [D, D], BF16, isOutput=False)
    out = nc.declare_dram_parameter("out", [512, D], F32, isOutput=True)

    v_local = nc.dram_tensor("v_local", [S, 512], BF16)
    v_ag = [nc.dram_tensor(f"v_ag{h}", [4, 1024, 512], BF16) for h in range(2)]
    # per-slab P partials, width-trimmed to kw = 512*(s+1)
    p_part = [nc.dram_tensor(f"p_part{s}", [512, 512 * (s + 1)], BF16) for s in range(4)]
    p_recv = [nc.dram_tensor(f"p_recv{s}", [128, 512 * (s + 1)], BF16) for s in range(4)]

    with tile.TileContext(nc) as tc:
        # persistent across phases
        qkp = tc.alloc_tile_pool(name="qk", bufs=1)
        qT = qkp.tile([128, 4, S], F32R)  # [dh-part, head, q]
        kT = qkp.tile([128, 4, S], F32R)
        otp = tc.alloc_tile_pool(name="otp", bufs=1)
        ot = otp.tile([128, NT, 512], BF16)  # [dv-part, dvt, own-q]
        with tc.tile_pool(name="const", bufs=1) as cst:
            ident = cst.tile([128, 128], F32)
            make_identity(nc, ident)
            ident_bf = cst.tile([128, 128], BF16)
            nc.vector.tensor_copy(out=ident_bf[:], in_=ident[:])
            # diag mask [128,128] f32: 0 where col <= row else -1e9
            dmask = cst.tile([128, 128], F32)
            nc.gpsimd.memset(dmask[:], 0.0)
            nc.gpsimd.affine_select(
                out=dmask[:],
                in_=dmask[:],
                compare_op=mybir.AluOpType.is_ge,
                fill=NEG,
                base=0,
                pattern=[[-1, 128]],
                channel_multiplier=1,
            )

            # ---------------- Phase A+B: projections, quarters ----------------
            with (
                tc.tile_pool(name="wsb", bufs=1) as wsb,
                tc.tile_pool(name="xq_pool", bufs=1) as xqp,
                tc.tile_pool(name="xbf_pool", bufs=4) as xbp,
                tc.tile_pool(name="drain", bufs=4) as drp,
                tc.tile_pool(name="ab_ps", bufs=8, space="PSUM") as pjp,
            ):
                wq_sb = wsb.tile([128, NT, 512], F32R)
                wk_sb = wsb.tile([128, NT, 512], F32R)
                wv_sb = wsb.tile([128, NT, 512], BF16)
                wq_src = wq.rearrange("(t p) c -> p t c", p=128)
                wk_src = wk.rearrange("(t p) c -> p t c", p=128)
                wv_src = wv.rearrange("(t p) c -> p t c", p=128)
                nc.sync.dma_start(wq_sb[:], wq_src)
                nc.sync.dma_start(wk_sb[:], wk_src)
                nc.sync.dma_start(wv_sb[:], wv_src)

                for qd in range(4):
                    s0 = qd * 512
                    xq = xqp.tile([128, NT, 512], F32R, tag="xq")
                    for Dt in range(NT):
                        nc.sync.dma_start(
                            xq[:, Dt, :], x[Dt * 128 : (Dt + 1) * 128, s0 : s0 + 512]
                        )
                    # q-pass then k-pass (psum banks 0-3 / 4-7 alternate by pool)
                    for dst, wsrc, eng in ((qT, wq_sb, "s"), (kT, wk_sb, "v")):
                        psums = [
                            pjp.tile([128, 512], F32, tag="ps", name=f"pj{_j}")
                            for _j in range(4)
                        ]
                        for Dt in range(NT):
                            for dt in range(4):
                                nc.tensor.matmul(
                                    psums[dt][:],
                                    wsrc[:, Dt, dt * 128 : (dt + 1) * 128],
                                    xq[:, Dt, :],
                                    start=(Dt == 0),
                                    stop=(Dt == NT - 1),
                                )
                        for dt in range(4):
                            if eng == "s":
                                nc.scalar.copy(
                                    out=dst[:, dt, s0 : s0 + 512], in_=psums[dt][:]
                                )
                            else:
                                nc.vector.tensor_copy(
                                    out=dst[:, dt, s0 : s0 + 512], in_=psums[dt][:]
                                )
                    # v-pass (bf16; xbf streamed; covers next-quarter xq WAR window)
                    psums = [
                        pjp.tile([128, 512], F32, tag="ps", name=f"pv{_j}")
                        for _j in range(4)
                    ]
                    for Dt in range(NT):
                        xb_t = xbp.tile([128, 512], BF16, tag="xb")
                        nc.sync.dma_start(
                            xb_t[:], xbf[Dt * 128 : (Dt + 1) * 128, s0 : s0 + 512]
                        )
                        for sb in range(4):
                            nc.tensor.matmul(
                                psums[sb][:],
                                xb_t[:, sb * 128 : (sb + 1) * 128],
                                wv_sb[:, Dt, :],
                                start=(Dt == 0),
                                stop=(Dt == NT - 1),
                            )
                    for sb in range(4):
                        v_sb = drp.tile([128, 512], BF16, tag="vsb")
                        nc.vector.tensor_copy(out=v_sb[:], in_=psums[sb][:])
                        r0 = s0 + sb * 128
                        nc.sync.dma_start(v_local[r0 : r0 + 128, :], v_sb[:])
                    if qd == 1 or qd == 3:
                        h = qd // 2
                        nc.gpsimd.collective_compute(
                            "AllGather",
                            mybir.AluOpType.bypass,
                            ins=[v_local[h * 1024 : (h + 1) * 1024, :]],
                            outs=[v_ag[h][:]],
                            replica_groups=GROUPS,
                        )

            # ---------------- Phase C + D interleaved ----------------
            with (
                tc.tile_pool(name="epool", bufs=2) as ep,
                tc.tile_pool(name="small", bufs=48) as smp,
                tc.tile_pool(name="dsm", bufs=8) as dsm,
                tc.tile_pool(name="psb", bufs=4) as psbp,
                tc.tile_pool(name="rp", bufs=1) as rp,
                tc.tile_pool(name="ptp", bufs=2) as ptp,
                tc.tile_pool(name="vfp", bufs=3) as vfp,
                tc.tile_pool(name="sc_ps", bufs=4, space="PSUM") as scp,
                tc.tile_pool(name="p_ps", bufs=2, space="PSUM") as ppp,
                tc.tile_pool(name="ot_ps", bufs=2, space="PSUM") as obp,
            ):
                def issue_scores(i):
                    """Score matmuls for q-tile i, all 4 heads. Returns ctx."""
                    s = i // 4
                    kwc = s + 1
                    vw = 128 * (i + 1)
                    dc = (i * 128) // 512
                    off = (i * 128) % 512
                    hctx = []
                    for h in range(4):
                        s_tiles = [
                            scp.tile([128, 512], F32, tag="S", name=f"sc{i}h{h}c{_c}")
                            for _c in range(kwc)
                        ]
                        for kc in range(kwc):
                            nc.tensor.matmul(
                                s_tiles[kc][:],
                                qT[:, h, i * 128 : (i + 1) * 128],
                                kT[:, h, kc * 512 : (kc + 1) * 512],
                                start=True,
                                stop=True,
                            )
                        hctx.append(s_tiles)
                    return (i, kwc, vw, dc, off, hctx)

                def issue_softmax(ctx, e_t):
                    """Mask, max, exp, rinv for all heads of tile i."""
                    i, kwc, vw, dc, off, hctx = ctx
                    kw = kwc * 512
                    d_hs = []
                    for h in range(4):
                        s_tiles = hctx[h]
                        nc.vector.tensor_tensor(
                            out=s_tiles[dc][:, off : off + 128],
                            in0=s_tiles[dc][:, off : off + 128],
                            in1=dmask[:],
                            op=mybir.AluOpType.add,
                        )
                        mx = None
                        for t in range(dc + 1):
                            w = min(vw - 512 * t, 512)
                            mxt = smp.tile([128, 1], F32, tag="mx")
                            nc.vector.reduce_max(
                                out=mxt[:],
                                in_=s_tiles[t][:, :w],
                                axis=mybir.AxisListType.X,
                            )
                            if mx is None:
                                mx = mxt
                            else:
                                mxn = smp.tile([128, 1], F32, tag="mx")
                                nc.vector.tensor_tensor(
                                    out=mxn[:], in0=mx[:], in1=mxt[:],
                                    op=mybir.AluOpType.max,
                                )
                                mx = mxn
                        nmS = smp.tile([128, 1], F32, tag="mx")
                        nc.vector.tensor_scalar_mul(nmS[:], mx[:], -SCALE)
                        rtot = None
                        for t in range(dc + 1):
                            w = min(vw - 512 * t, 512)
                            rc = smp.tile([128, 1], F32, tag="mx")
                            nc.scalar.activation(
                                out=e_t[:, h, 512 * t : 512 * t + w],
                                in_=s_tiles[t][:, :w],
                                func=mybir.ActivationFunctionType.Exp,
                                bias=nmS[:],
                                scale=SCALE,
                                accum_out=rc[:],
                            )
                            if rtot is None:
                                rtot = rc
                            else:
                                rn = smp.tile([128, 1], F32, tag="mx")
                                nc.vector.tensor_tensor(
                                    out=rn[:], in0=rtot[:], in1=rc[:],
                                    op=mybir.AluOpType.add,
                                )
                                rtot = rn
                        if vw < kw:
                            nc.gpsimd.memset(e_t[:, h, vw:kw], 0.0)
                        rinv = smp.tile([128, 1], F32, tag="mx")
                        nc.vector.reciprocal(out=rinv[:], in_=rtot[:])
                        d_h = dsm.tile([128, 128], BF16, tag="D")
                        nc.vector.tensor_scalar_mul(d_h[:], ident_bf[:], rinv[:])
                        d_hs.append(d_h)
                    return d_hs

                def issue_p(ctx, e_t, d_hs):
                    i, kwc, vw, dc, off, hctx = ctx
                    s = i // 4
                    for kc in range(kwc):
                        p_t = ppp.tile([128, 512], F32, tag="P")
                        for h in range(4):
                            nc.tensor.matmul(
                                p_t[:],
                                d_hs[h][:],
                                e_t[:, h, kc * 512 : (kc + 1) * 512],
                                start=(h == 0),
                                stop=(h == 3),
                            )
                        pc = psbp.tile([128, 512], BF16, tag="psb")
                        nc.scalar.copy(out=pc[:], in_=p_t[:])
                        r0 = (i - 4 * s) * 128
                        nc.sync.dma_start(
                            p_part[s][r0 : r0 + 128, kc * 512 : (kc + 1) * 512],
                            pc[:],
                        )

                def issue_slab_C(s):
                    """Scores+softmax+P for slab s with 1-deep pipeline."""
                    iis = [4 * s + j for j in range(4)]
                    pend = None  # (ctx, e_t, d_hs)
                    for i in iis:
                        ctx = issue_scores(i)
                        if pend is not None:
                            issue_p(*pend)
                            pend = None
                        e_t = ep.tile([128, 4, 2048], BF16, tag="E")
                        d_hs = issue_softmax(ctx, e_t)
                        pend = (ctx, e_t, d_hs)
                    issue_p(*pend)
                    nc.gpsimd.collective_compute(
                        "ReduceScatter",
                        mybir.AluOpType.add,
                        ins=[p_part[s][:]],
                        outs=[p_recv[s][:]],
                        replica_groups=GROUPS,
                    )

                def issue_slab_D(s):
                    """Load P_own, transpose to pt, OT for slab s."""
                    kw = 512 * (s + 1)
                    nkt = 4 * (s + 1)
                    pown_bf = rp.tile([128, 2048], BF16, tag="POW")
                    nc.sync.dma_start(pown_bf[:, :kw], p_recv[s][:])
                    pown = rp.tile([128, 2048], F32, tag="POWF")
                    nc.vector.tensor_copy(out=pown[:, :kw], in_=pown_bf[:, :kw])
                    pt = ptp.tile([128, NT, 128], BF16, tag="PT")
                    for kg in range(nkt // 4):  # transpose 4 k-tiles per psum tile
                        tr = obp.tile([128, 4, 128], F32, tag="OTB", name=f"tr{s}_{kg}")
                        for j in range(4):
                            kt = 4 * kg + j
                            nc.tensor.transpose(
                                tr[:, j, :],
                                pown[:, kt * 128 : (kt + 1) * 128],
                                ident[:],
                            )
                        nc.vector.tensor_copy(
                            out=pt[:, 4 * kg : 4 * kg + 4, :], in_=tr[:]
                        )
                    for dg in range(4):  # groups of 4 dvt share one psum tile
                        po = obp.tile([128, 4, 128], F32, tag="OTB", name=f"po{s}_{dg}")
                        for j in range(4):
                            dvt = 4 * dg + j
                            vf = vfp.tile([128, NT, 128], BF16, tag="VF")
                            g, cg = dvt // 4, (dvt % 4) * 128
                            for h in range(2):
                                if 8 * h >= nkt:
                                    break
                                vsrc = v_ag[h][g].rearrange("(t p) d -> p t d", p=128)
                                nhk = min(nkt - 8 * h, 8)
                                nc.sync.dma_start(
                                    vf[:, 8 * h : 8 * h + nhk, :],
                                    vsrc[:, :nhk, cg : cg + 128],
                                )
                            for kt in range(nkt):
                                nc.tensor.matmul(
                                    po[:, j, :],
                                    vf[:, kt, :],
                                    pt[:, kt, :],
                                    start=(kt == 0),
                                    stop=(kt == nkt - 1),
                                )
                        if dg % 2 == 0:
                            nc.scalar.copy(
                                out=ot[:, 4 * dg : 4 * dg + 4, s * 128 : (s + 1) * 128],
                                in_=po[:],
                            )
                        else:
                            nc.vector.tensor_copy(
                                out=ot[:, 4 * dg : 4 * dg + 4, s * 128 : (s + 1) * 128],
                                in_=po[:],
                            )

                issue_slab_C(3)
                issue_slab_C(2)
                issue_slab_C(1)
                issue_slab_D(3)
                issue_slab_C(0)
                issue_slab_D(2)
                issue_slab_D(1)
                issue_slab_D(0)

        # ---------------- Phase Y ----------------
        with (
            tc.tile_pool(name="wop", bufs=4) as wop,
            tc.tile_pool(name="ysb", bufs=4) as ysbp,
            tc.tile_pool(name="y_ps", bufs=8, space="PSUM") as yps,
        ):
            for nh in range(2):  # output col halves [0,1024), [1024,2048)
                c0 = nh * 1024
                yp = [
                    yps.tile([128, 512], F32, tag="Y", name=f"y{nh}_{_j}")
                    for _j in range(8)
                ]
                for dvt in range(NT):
                    wo_t = wop.tile([128, 1024], BF16, tag="wo")
                    nc.sync.dma_start(
                        wo_t[:], wo[dvt * 128 : (dvt + 1) * 128, c0 : c0 + 1024]
                    )
                    for j in range(8):
                        qb, nc2 = divmod(j, 2)
                        nc.tensor.matmul(
                            yp[j][:],
                            ot[:, dvt, qb * 128 : (qb + 1) * 128],
                            wo_t[:, nc2 * 512 : (nc2 + 1) * 512],
                            start=(dvt == 0),
                            stop=(dvt == NT - 1),
                        )
                for j in range(8):
                    qb, nc2 = divmod(j, 2)
                    y_sb = ysbp.tile([128, 512], F32, tag="ysb")
                    if j % 2 == 0:
                        nc.scalar.copy(out=y_sb[:], in_=yp[j][:])
                    else:
                        nc.vector.tensor_copy(out=y_sb[:], in_=yp[j][:])
                    nc.sync.dma_start(
                        out[
                            qb * 128 : (qb + 1) * 128,
                            c0 + nc2 * 512 : c0 + nc2 * 512 + 512,
                        ],
                        y_sb[:],
                    )
        otp.release()
        qkp.release()

    nc.compile()
    return nc


_NC_CACHE = None


def kernel(x, W_q, W_k, W_v, W_o):
    global _NC_CACHE
    x = np.asarray(x, dtype=np.float32)
    W_q = np.asarray(W_q, dtype=np.float32)
    W_k = np.asarray(W_k, dtype=np.float32)
    W_v = np.asarray(W_v, dtype=np.float32)
    W_o = np.asarray(W_o, dtype=np.float32)
    if _NC_CACHE is None:
        _NC_CACHE = build()
    nc = _NC_CACHE

    wo_bf = W_o.astype(ml_dtypes.bfloat16)
    xT = [np.ascontiguousarray(x[g].T) for g in range(2)]
    xT_bf = [t.astype(ml_dtypes.bfloat16) for t in xT]
    in_maps = []
    for c in range(8):
        g, r = divmod(c, 4)
        in_maps.append(
            {
                "x": xT[g],
                "xbf": xT_bf[g],
                "wq": np.ascontiguousarray(W_q[:, 512 * r : 512 * (r + 1)]),
                "wk": np.ascontiguousarray(W_k[:, 512 * r : 512 * (r + 1)]),
                "wv": np.ascontiguousarray(W_v[:, 512 * r : 512 * (r + 1)]).astype(ml_dtypes.bfloat16),
                "wo": wo_bf,
            }
        )
    res = run_bass_kernel_spmd(nc, in_maps, core_ids=list(range(8)))
    Y = np.empty((2, S, D), dtype=np.float32)
    for c in range(8):
        g, r = divmod(c, 4)
        o = res.results[c]["out"]
        for s_idx in range(4):
            t = 4 * s_idx + r
            Y[g, t * 128 : (t + 1) * 128, :] = o[s_idx * 128 : (s_idx + 1) * 128, :]
    return Y
